# revision 8
# baseline (speedup 1.0000x reference)
"""3D Haar wavelet transform (2x2x2, causal temporal pad) on 8 Trainium2 cores.

Input  x: (2, 3, 33, 512, 512) fp32
Output y: (2, 24, 17, 256, 256) fp32   (channel = 3*s + c, s = subband)

Sharding: pure data parallel over H — core ci handles input rows
[64*ci, 64*ci+64) i.e. output rows [32*ci, 32*ci+32).

Memory-bound problem -> move I/O in bf16 (rel err ~2e-3, gate is 2e-2),
halving HBM traffic vs fp32 (~74 us roofline at 358 GB/s per core).

All THREE Haar stages (T, H, W) are fused into a single 128x128 matmul
by packing the 2x2x2 block offsets into the partition dim on the host:
  partition p = i*64 + j*32 + k*16 + r   (i=temporal, j=row, k=col parity,
                                          r = q%16 of the 32 output rows)
  free     f = T'*512 + qh*256 + w'      (qh = q//16)
Weight W[p, m] = (-1)^(i*di + j*dj + k*dk) * [r_p == r_m],
  m = di*64 + dj*32 + dk*16 + r  (bf16-exact +-1; scale 0.3536 applied
  on the PSUM->SBUF evacuation, split across ACT and DVE engines).

The causal pad (frame -1 = frame 0) is NOT transferred: at T'=0 the
temporal-diff subbands (di=1) are identically zero and the di=0 ones are
2*frame0 contributions, so T'=0 uses a K=64 matmul over the i=1
partitions with a folded +-2 weight block. Input arrives as two tensors
(x0: odd frames for i=0 partitions, 16 T'; x1: even frames, 17 T').

Per-core device pipeline, per (b, c) slab ([128, 8704] bf16):
  4 in-DMAs (sync/HWDGE, 64-partition halves on disjoint SBUF ports)
  17 matmuls [128, 512] bf16 -> PSUM fp32 (4-bank groups x2)
  5 evacuations x0.3536 -> bf16 staging (alternating ACT / DVE)
  2 out-DMAs (scalar/HWDGE, fully contiguous rows)
Host does all index packing/unpacking (prep/post transposes + bf16 cast).
"""

import numpy as np
import ml_dtypes

import concourse.bacc as bacc
import concourse.mybir as mybir
from concourse import tile
from concourse.bass_utils import run_bass_kernel_spmd

BF16 = ml_dtypes.bfloat16

P = 128
B_, C_, T_, H_, W_ = 2, 3, 33, 512, 512
NCORES = 8
HC = H_ // NCORES          # 64 input rows per core
TP = (T_ + 1) // 2         # 17 output frames
HP = HC // 2               # 32 output rows per core
WP = W_ // 2               # 256 output cols
F = TP * 512               # 8704 free columns per (b, c) slab
F0 = (TP - 1) * 512        # 8192 free columns for the i=0 (odd-frame) half
FH = 9 * 512               # in-DMA split point (4608)
SCALE = float(np.float32(0.3536))
F32 = mybir.dt.float32
BF16DT = mybir.dt.bfloat16


def _haar_weights() -> np.ndarray:
    """[128, 256] bf16: cols 0:128 = full +-1 W; cols 128:256 rows 64:128 =
    folded T'=0 block (+-2, di=0 outputs only; pad frame == frame 0)."""
    W = np.zeros((P, 2 * P), dtype=np.float32)
    for i in range(2):
        for j in range(2):
            for k in range(2):
                p0 = i * 64 + j * 32 + k * 16
                for di in range(2):
                    for dj in range(2):
                        for dk in range(2):
                            m0 = di * 64 + dj * 32 + dk * 16
                            sgn = (-1.0) ** (i * di + j * dj + k * dk)
                            for r in range(16):
                                W[p0 + r, m0 + r] = sgn
                                if i == 1 and di == 0:
                                    W[p0 + r, P + m0 + r] = 2.0 * sgn
    return W.astype(BF16)


def build_nc():
    nc = bacc.Bacc("TRN2", target_bir_lowering=False, debug=False)
    # [b, c, p, (T', qh, w')]; the [0:64, 0:512) corner (pad frame slot for
    # the i=0 partitions) is dead — host leaves it unfilled, kernel skips it.
    x_d = nc.dram_tensor("x", [B_, C_, P, F], BF16DT, kind="ExternalInput")
    y_d = nc.dram_tensor("y", [B_, C_, P, F], BF16DT, kind="ExternalOutput")
    w_d = nc.inline_tensor(_haar_weights(), name="haar_w")

    with tile.TileContext(nc) as tc:
        with (
            tc.tile_pool(name="wpool", bufs=1) as wpool,
            tc.tile_pool(name="apool", bufs=3) as apool,
            tc.tile_pool(name="cpool", bufs=3) as cpool,
            tc.tile_pool(name="psum", bufs=2, space="PSUM") as psum_pool,
        ):
            # gpsimd/SWDGE: keeps the sync HWDGE ring free for input DMAs
            w_sb = wpool.tile([P, 2 * P], BF16DT)
            nc.gpsimd.dma_start(out=w_sb[:], in_=w_d[:])

            last = B_ * C_ - 1
            for bc in range(B_ * C_):
                b, c = divmod(bc, C_)
                xin = x_d[b, c]
                yout = y_d[b, c]
                a = apool.tile([P, F], BF16DT, tag="a")
                # tiny T'=0 block (i=1 rows only) first: unblocks the
                # K=64 matmul ASAP; the big DMAs stay full-128-partition
                # (64-partition transfers halve SBUF-port bandwidth).
                nc.sync.dma_start(out=a[64:128, 0:512], in_=xin[64:128, 0:512])
                nc.sync.dma_start(out=a[:, 512:FH], in_=xin[:, 512:FH])
                nc.sync.dma_start(out=a[:, FH:], in_=xin[:, FH:])
                cb = cpool.tile([P, F], BF16DT, tag="c")
                for gi, g0 in enumerate(range(0, TP, 4)):
                    tg = min(4, TP - g0)
                    ps = psum_pool.tile([P, 2048], F32, tag="ps")
                    # dummy matmuls (overwritten by the real start=True
                    # ones): keep the PE HAM activity window busy so real
                    # matmuls run at 2.4 GHz instead of the cold 1.2 GHz.
                    # Skipped on the 1-matmul tail group (critical path).
                    for _ in range(0 if tg < 4 else 6):
                        nc.tensor.matmul(
                            ps[:, 0:256],
                            w_sb[:, 0:P],
                            w_sb[:, 0 : 2 * P],
                            start=True,
                            stop=True,
                        )
                    for t in range(tg):
                        if g0 + t == 0:
                            nc.tensor.matmul(
                                ps[:, 0:512],
                                w_sb[64:128, P : 2 * P],
                                a[64:128, 0:512],
                                start=True,
                                stop=True,
                            )
                        else:
                            nc.tensor.matmul(
                                ps[:, t * 512 : (t + 1) * 512],
                                w_sb[:, 0:P],
                                a[:, (g0 + t) * 512 : (g0 + t + 1) * 512],
                                start=True,
                                stop=True,
                            )
                    src = ps[:, : tg * 512]
                    dst = cb[:, g0 * 512 : (g0 + tg) * 512]
                    # balance evacuation: ACT groups {0,2}, DVE {1,3,4}
                    if gi % 2 == 0 and tg == 4:
                        nc.scalar.mul(dst, src, SCALE)
                    else:
                        nc.vector.tensor_scalar_mul(dst, src, SCALE)
                    # drain staging as soon as its groups are complete;
                    # small tapered last chunk keeps the tail short
                    if g0 + tg == 8:
                        nc.scalar.dma_start(
                            out=yout[:, : 8 * 512], in_=cb[:, : 8 * 512]
                        )
                    elif g0 + tg == 16:
                        nc.scalar.dma_start(
                            out=yout[:, 8 * 512 : 16 * 512],
                            in_=cb[:, 8 * 512 : 16 * 512],
                        )
                    elif g0 + tg == TP:
                        eng = nc.sync if bc == last else nc.scalar
                        eng.dma_start(
                            out=yout[:, 16 * 512 :], in_=cb[:, 16 * 512 :]
                        )
    nc.compile()
    return nc


_NC_CACHE = None


def _get_nc():
    global _NC_CACHE
    if _NC_CACHE is None:
        _NC_CACHE = build_nc()
    return _NC_CACHE


def _prep_core_input(x16: np.ndarray, ci: int):
    """x16 (full input, bf16) -> {x: [B,C,128,8704]} (pad corner unfilled)."""
    rows = slice(HC * ci, HC * (ci + 1))
    xa = np.empty((B_, C_, P, F), dtype=BF16)
    for frames, ntp, prng, cols in (
        (slice(1, None, 2), TP - 1, slice(0, 64), slice(512, None)),   # i=0
        (slice(0, None, 2), TP, slice(64, 128), slice(0, None)),       # i=1
    ):
        xc = x16[:, :, frames, rows, :]                  # [2,3,ntp,64,512]
        # h = qh*32 + r*2 + j ; w -> (w', k)
        xc = xc.reshape(B_, C_, ntp, 2, 16, 2, WP, 2)    # [b,c,T',qh,r,j,w',k]
        xc = xc.transpose(0, 1, 5, 7, 4, 2, 3, 6)        # [b,c,j,k,r,T',qh,w']
        xa[:, :, prng, cols] = xc.reshape(B_, C_, 64, ntp * 512)
    return {"x": xa}


def _make_in_maps(x: np.ndarray):
    x16 = np.asarray(x, dtype=np.float32).astype(BF16)
    return [_prep_core_input(x16, ci) for ci in range(NCORES)]


def kernel(x: np.ndarray) -> np.ndarray:
    assert x.shape == (B_, C_, T_, H_, W_), x.shape
    nc = _get_nc()
    in_maps = _make_in_maps(x)
    res = run_bass_kernel_spmd(nc, in_maps, core_ids=list(range(NCORES)))
    y = np.empty((B_, 8 * C_, TP, H_ // 2, WP), dtype=np.float32)
    for ci in range(NCORES):
        yc = np.asarray(res.results[ci]["y"])            # [2,3,128,8704] bf16
        yc = yc.reshape(B_, C_, 2, 2, 2, 16, TP, 2, WP)  # [b,c,di,dj,dk,r,T',qh,w']
        yc = yc.transpose(0, 2, 3, 4, 1, 6, 7, 5, 8)     # [b,di,dj,dk,c,T',qh,r,w']
        y[:, :, :, HP * ci : HP * (ci + 1), :] = yc.reshape(B_, 8 * C_, TP, HP, WP)
    return y


# revision 9
# speedup vs baseline: 1.1808x; 1.1808x over previous
"""3D Haar wavelet transform (2x2x2, causal temporal pad) on 8 Trainium2 cores.

Input  x: (2, 3, 33, 512, 512) fp32
Output y: (2, 24, 17, 256, 256) fp32   (channel = 3*s + c, s = subband)

Sharding: pure data parallel over H — core ci handles input rows
[64*ci, 64*ci+64) i.e. output rows [32*ci, 32*ci+32).

Memory-bound problem -> move I/O in bf16 (rel err ~2e-3, gate is 2e-2),
halving HBM traffic vs fp32 (~74 us roofline at 358 GB/s per core).

All THREE Haar stages (T, H, W) are fused into a single 128x128 matmul
by packing the 2x2x2 block offsets into the partition dim on the host:
  partition p = i*64 + j*32 + k*16 + r   (i=temporal, j=row, k=col parity,
                                          r = q%16 of the 32 output rows)
  free     f = T'*512 + qh*256 + w'      (qh = q//16)
Weight W[p, m] = (-1)^(i*di + j*dj + k*dk) * [r_p == r_m],
  m = di*64 + dj*32 + dk*16 + r  (bf16-exact +-1; scale 0.3536 applied
  on the PSUM->SBUF evacuation, split across ACT and DVE engines).

The causal pad (frame -1 = frame 0) is NOT transferred: at T'=0 the
temporal-diff subbands (di=1) are identically zero and the di=0 ones are
2*frame0 contributions, so T'=0 uses a K=64 matmul over the i=1
partitions with a folded +-2 weight block. Input arrives as two tensors
(x0: odd frames for i=0 partitions, 16 T'; x1: even frames, 17 T').

Per-core device pipeline, per (b, c) slab ([128, 8704] bf16):
  4 in-DMAs (sync/HWDGE, 64-partition halves on disjoint SBUF ports)
  17 matmuls [128, 512] bf16 -> PSUM fp32 (4-bank groups x2)
  5 evacuations x0.3536 -> bf16 staging (alternating ACT / DVE)
  2 out-DMAs (scalar/HWDGE, fully contiguous rows)
Host does all index packing/unpacking (prep/post transposes + bf16 cast).
"""

import numpy as np
import ml_dtypes

import concourse.bacc as bacc
import concourse.mybir as mybir
from concourse import tile
from concourse.bass_utils import run_bass_kernel_spmd

BF16 = ml_dtypes.bfloat16

P = 128
B_, C_, T_, H_, W_ = 2, 3, 33, 512, 512
NCORES = 8
HC = H_ // NCORES          # 64 input rows per core
TP = (T_ + 1) // 2         # 17 output frames
HP = HC // 2               # 32 output rows per core
WP = W_ // 2               # 256 output cols
F = TP * 512               # 8704 free columns per (b, c) slab
F0 = (TP - 1) * 512        # 8192 free columns for the i=0 (odd-frame) half
FH = 9 * 512               # in-DMA split point (4608)
SCALE = float(np.float32(0.3536))
F32 = mybir.dt.float32
BF16DT = mybir.dt.bfloat16


def _haar_weights() -> np.ndarray:
    """[128, 256] bf16: cols 0:128 = full +-1 W; cols 128:256 rows 64:128 =
    folded T'=0 block (+-2, di=0 outputs only; pad frame == frame 0)."""
    W = np.zeros((P, 2 * P), dtype=np.float32)
    for i in range(2):
        for j in range(2):
            for k in range(2):
                p0 = i * 64 + j * 32 + k * 16
                for di in range(2):
                    for dj in range(2):
                        for dk in range(2):
                            m0 = di * 64 + dj * 32 + dk * 16
                            sgn = (-1.0) ** (i * di + j * dj + k * dk)
                            for r in range(16):
                                W[p0 + r, m0 + r] = sgn
                                if i == 1 and di == 0:
                                    W[p0 + r, P + m0 + r] = 2.0 * sgn
    return W.astype(BF16)


def build_nc():
    nc = bacc.Bacc("TRN2", target_bir_lowering=False, debug=False)
    # [b, c, p, (T', qh, w')]; the [0:64, 0:512) corner (pad frame slot for
    # the i=0 partitions) is dead — host leaves it unfilled, kernel skips it.
    x_d = nc.dram_tensor("x", [B_, C_, P, F], BF16DT, kind="ExternalInput")
    y_d = nc.dram_tensor("y", [B_, C_, P, F], BF16DT, kind="ExternalOutput")
    w_d = nc.inline_tensor(_haar_weights(), name="haar_w")

    with tile.TileContext(nc) as tc:
        with (
            tc.tile_pool(name="wpool", bufs=1) as wpool,
            tc.tile_pool(name="apool", bufs=3) as apool,
            tc.tile_pool(name="cpool", bufs=3) as cpool,
            tc.tile_pool(name="psum", bufs=2, space="PSUM") as psum_pool,
        ):
            # gpsimd/SWDGE: keeps the sync HWDGE ring free for input DMAs
            w_sb = wpool.tile([P, 2 * P], BF16DT)
            nc.gpsimd.dma_start(out=w_sb[:], in_=w_d[:])

            last = B_ * C_ - 1
            for bc in range(B_ * C_):
                b, c = divmod(bc, C_)
                xin = x_d[b, c]
                yout = y_d[b, c]
                a = apool.tile([P, F], BF16DT, tag="a")
                # tiny T'=0 block (i=1 rows only) first: unblocks the
                # K=64 matmul ASAP; the big DMAs stay full-128-partition
                # (64-partition transfers halve SBUF-port bandwidth).
                nc.sync.dma_start(out=a[64:128, 0:512], in_=xin[64:128, 0:512])
                nc.sync.dma_start(out=a[:, 512:FH], in_=xin[:, 512:FH])
                nc.sync.dma_start(out=a[:, FH:], in_=xin[:, FH:])
                cb = cpool.tile([P, F], BF16DT, tag="c")
                for gi, g0 in enumerate(range(0, TP, 4)):
                    tg = min(4, TP - g0)
                    ps = psum_pool.tile([P, 2048], F32, tag="ps")
                    for t in range(tg):
                        if g0 + t == 0:
                            nc.tensor.matmul(
                                ps[:, 0:512],
                                w_sb[64:128, P : 2 * P],
                                a[64:128, 0:512],
                                start=True,
                                stop=True,
                            )
                        else:
                            nc.tensor.matmul(
                                ps[:, t * 512 : (t + 1) * 512],
                                w_sb[:, 0:P],
                                a[:, (g0 + t) * 512 : (g0 + t + 1) * 512],
                                start=True,
                                stop=True,
                            )
                    src = ps[:, : tg * 512]
                    dst = cb[:, g0 * 512 : (g0 + tg) * 512]
                    # balance evacuation: ACT groups {0,2}, DVE {1,3,4}
                    if gi % 2 == 0 and tg == 4:
                        nc.scalar.mul(dst, src, SCALE)
                    else:
                        nc.vector.tensor_scalar_mul(dst, src, SCALE)
                    # drain staging as soon as its groups are complete;
                    # small tapered last chunk keeps the tail short
                    if g0 + tg == 8:
                        nc.scalar.dma_start(
                            out=yout[:, : 8 * 512], in_=cb[:, : 8 * 512]
                        )
                    elif g0 + tg == 16:
                        nc.scalar.dma_start(
                            out=yout[:, 8 * 512 : 16 * 512],
                            in_=cb[:, 8 * 512 : 16 * 512],
                        )
                    elif g0 + tg == TP:
                        eng = nc.sync if bc == last else nc.scalar
                        eng.dma_start(
                            out=yout[:, 16 * 512 :], in_=cb[:, 16 * 512 :]
                        )
    nc.compile()
    return nc


_NC_CACHE = None


def _get_nc():
    global _NC_CACHE
    if _NC_CACHE is None:
        _NC_CACHE = build_nc()
    return _NC_CACHE


def _prep_core_input(x16: np.ndarray, ci: int):
    """x16 (full input, bf16) -> {x: [B,C,128,8704]} (pad corner unfilled)."""
    rows = slice(HC * ci, HC * (ci + 1))
    xa = np.empty((B_, C_, P, F), dtype=BF16)
    for frames, ntp, prng, cols in (
        (slice(1, None, 2), TP - 1, slice(0, 64), slice(512, None)),   # i=0
        (slice(0, None, 2), TP, slice(64, 128), slice(0, None)),       # i=1
    ):
        xc = x16[:, :, frames, rows, :]                  # [2,3,ntp,64,512]
        # h = qh*32 + r*2 + j ; w -> (w', k)
        xc = xc.reshape(B_, C_, ntp, 2, 16, 2, WP, 2)    # [b,c,T',qh,r,j,w',k]
        xc = xc.transpose(0, 1, 5, 7, 4, 2, 3, 6)        # [b,c,j,k,r,T',qh,w']
        xa[:, :, prng, cols] = xc.reshape(B_, C_, 64, ntp * 512)
    return {"x": xa}


def _make_in_maps(x: np.ndarray):
    x16 = np.asarray(x, dtype=np.float32).astype(BF16)
    return [_prep_core_input(x16, ci) for ci in range(NCORES)]


def kernel(x: np.ndarray) -> np.ndarray:
    assert x.shape == (B_, C_, T_, H_, W_), x.shape
    nc = _get_nc()
    in_maps = _make_in_maps(x)
    res = run_bass_kernel_spmd(nc, in_maps, core_ids=list(range(NCORES)))
    y = np.empty((B_, 8 * C_, TP, H_ // 2, WP), dtype=np.float32)
    for ci in range(NCORES):
        yc = np.asarray(res.results[ci]["y"])            # [2,3,128,8704] bf16
        yc = yc.reshape(B_, C_, 2, 2, 2, 16, TP, 2, WP)  # [b,c,di,dj,dk,r,T',qh,w']
        yc = yc.transpose(0, 2, 3, 4, 1, 6, 7, 5, 8)     # [b,di,dj,dk,c,T',qh,r,w']
        y[:, :, :, HP * ci : HP * (ci + 1), :] = yc.reshape(B_, 8 * C_, TP, HP, WP)
    return y
